# revision 1
# baseline (speedup 1.0000x reference)
# Trainium2 Bass kernel for AtomTypeGNN message passing.
#
#   adj_exp[m,k] = sum_n dist_adj[m,n] * dist_exp[m,n,k]          (streams 1 GiB)
#   feat[m,o]    = sum_{f,h} adj_exp[m,f] * w[f,h,o] * emb[m,h]
#   out          = softplus(feat) + b
#
# Output row m depends only on row m of the inputs -> pure data parallel over
# atoms, 8 NeuronCores, 256 atoms each, no collectives.
#
# Device strategy (per core):
#   Step 1: for each atom m, 16 accumulating matmuls contract n in chunks of
#     128: stationary = adj chunk [128,1] (bf16), moving = exp tile [128,64]
#     (bf16), accumulating adj_exp[m,:] into a PSUM bank (one bank per atom:
#     start=True clears the whole bank's has_written bits).  Partition p holds
#     n = 16p + c for chunk c, so a 4-atom group is one perfectly natural
#     [128, 4096] DMA with 8 KiB contiguous per partition.
#   Step 2: feat[m,o] = sum_f adj_exp[m,f] * G_f[m,o] with G_f = emb @ w[f]
#     precomputed on the PE during the stream (embT stationary, w[f] moving)
#     and cast to bf16 in SBUF; VectorE scalar_tensor_tensor chains the
#     scale-accumulate over f at each block boundary.
#
#   Queue discipline matters: the exp stream owns nc.sync (SP HWDGE); psum
#   evacuation copies own ScalarE; G-casts + the stt chain own VectorE;
#   scratch/const/out DMAs ride gpsimd (SWDGE) so no stream-side engine FIFO
#   ever blocks behind a long cross-phase semaphore wait.
#   adj_exp transits PSUM -> SBUF (ScalarE copy) -> DRAM scratch -> SBUF with
#   atoms on partitions (DMA cannot touch PSUM and engines cannot cross
#   partitions).
#
# Inputs are pre-swizzled/cast on the host (bf16 compute, f32 accumulate:
# ~3e-3 relative error, memory roofline halves to ~190us/core).

import numpy as np
import ml_dtypes

N = 2048
K = 64
H = 128
OUT = 128
N_CORES = 8
M = N // N_CORES  # 256 atoms per core

_BF = ml_dtypes.bfloat16

_CACHE = {}


def _ensure_path():
    import sys

    for p in ("/opt/trn_rl_repo",):
        if p not in sys.path:
            sys.path.insert(0, p)


def _build():
    _ensure_path()
    import concourse.bass as bass  # noqa: F401
    import concourse.tile as tile
    from concourse import bacc, mybir

    f32 = mybir.dt.float32
    bf16 = mybir.dt.bfloat16

    nc = bacc.Bacc(
        "TRN2",
        target_bir_lowering=False,
        debug=False,
        num_devices=N_CORES,
    )

    # [t, p, mq]: atom group t = atoms 4t..4t+3, partition p, mq = 1024*mm + q,
    # q = 64*c + k, n = 16p + c.  Per partition 8 KiB contiguous in DRAM.
    exp_d = nc.declare_dram_parameter("exp", [M // 4, 128, 4096], bf16, isOutput=False)
    # adjA[j, 16m + c] = dist_adj[m, 16j + c]
    adjA_d = nc.declare_dram_parameter("adjA", [128, 16 * M], bf16, isOutput=False)
    # embT[h, m]
    embT_d = nc.declare_dram_parameter("embT", [H, M], bf16, isOutput=False)
    # w2[h, 128f + o] = bilinear_w[f, h, o]
    w_d = nc.declare_dram_parameter("w", [H, K * OUT], bf16, isOutput=False)
    # bias broadcast to all partitions
    bias_d = nc.declare_dram_parameter("bias", [128, OUT], f32, isOutput=False)
    out_d = nc.declare_dram_parameter("out", [M, OUT], f32, isOutput=True)

    # adj_exp bounce buffer, row t = atoms 4t..4t+3
    scratch_d = nc.dram_tensor("scratch", [M // 4, 4 * K], f32)
    scratch_rows = scratch_d[:].rearrange("t (a f) -> (t a) f", f=K)  # [M, K]

    with tile.TileContext(nc) as tc:
        with (
            tc.tile_pool(name="const", bufs=1) as constp,
            tc.tile_pool(name="exp", bufs=8) as expp,
            tc.tile_pool(name="ps1", bufs=5, space="PSUM") as ps1p,
            tc.tile_pool(name="stage", bufs=4) as stagep,
            tc.tile_pool(name="aexp", bufs=2) as aexpp,
            tc.tile_pool(name="ps2", bufs=3, space="PSUM") as ps2p,
            tc.tile_pool(name="gsb", bufs=2) as gsbp,
            tc.tile_pool(name="acc", bufs=2) as accp,
            tc.tile_pool(name="outp", bufs=4) as outp,
            tc.tile_pool(name="ybuf", bufs=1) as ybufp,
        ):
            adjA = constp.tile([128, 16 * M], bf16, tag="adjA")
            nc.gpsimd.dma_start(adjA[:], adjA_d[:, :])
            wsb = constp.tile([128, K * OUT], bf16, tag="wsb")
            nc.gpsimd.dma_start(wsb[:], w_d[:, :])
            embT = constp.tile([128, M], bf16, tag="embT")
            nc.gpsimd.dma_start(embT[:], embT_d[:, :])
            biassb = constp.tile([128, OUT], f32, tag="bias")
            nc.gpsimd.dma_start(biassb[:], bias_d[:, :])

            for blk in range(M // 128):
                # G_f = (emb @ w[f]) for this block, computed during the
                # stream (fills PE/ACT bubbles), cast to bf16 in SBUF so the
                # tail scale-accumulate chain is cheap.
                gsb = gsbp.tile([128, K * OUT], bf16, tag="gsb")

                for mg in range(32):  # groups of 4 atoms, one DMA each
                    m0 = blk * 128 + mg * 4
                    et = expp.tile([128, 4096], bf16, tag="exp")
                    nc.sync.dma_start(et[:], exp_d[blk * 32 + mg])
                    stg = stagep.tile([1, 4 * K], mybir.dt.float32, tag="stage")
                    for mm in range(4):
                        m = m0 + mm
                        # own PSUM bank per atom: start=True clears whole bank
                        ps = ps1p.tile([128, K], mybir.dt.float32, tag="ps1")
                        for c in range(16):
                            nc.tensor.matmul(
                                ps[0:1, :],
                                adjA[:, 16 * m + c : 16 * m + c + 1],
                                et[:, 1024 * mm + K * c : 1024 * mm + K * (c + 1)],
                                start=(c == 0),
                                stop=(c == 15),
                            )
                        nc.scalar.copy(stg[0:1, K * mm : K * (mm + 1)], ps[0:1, :])
                    nc.gpsimd.dma_start(
                        scratch_d[blk * 32 + mg : blk * 32 + mg + 1, :], stg[0:1, :]
                    )
                    # interleave two G_f matmuls per atom group
                    for f in (2 * mg, 2 * mg + 1):
                        g2 = ps2p.tile([128, OUT], mybir.dt.float32, tag="ps2")
                        nc.tensor.matmul(
                            g2[:, :],
                            embT[:, 128 * blk : 128 * (blk + 1)],
                            wsb[:, OUT * f : OUT * (f + 1)],
                            start=True,
                            stop=True,
                        )
                        nc.vector.tensor_copy(gsb[:, OUT * f : OUT * (f + 1)], g2[:, :])

                # ---- step 2 tail for this block of 128 atoms ----
                aexp = aexpp.tile([128, K], mybir.dt.float32, tag="aexp")
                nc.sync.dma_start(
                    aexp[:], scratch_rows[128 * blk : 128 * (blk + 1), :]
                )
                # last block: split the scale-accumulate between the DVE
                # chain (f < NCH) and independent ScalarE scaled-copies
                # (f >= NCH) folded by one DVE tensor_reduce -- they overlap,
                # shortening the tail by ~8us.
                NCH = 36 if blk == M // 128 - 1 else K
                acc = None
                for f in range(NCH):
                    nacc = accp.tile([128, OUT], mybir.dt.float32, tag="acc")
                    if f == 0:
                        nc.vector.tensor_scalar_mul(
                            nacc[:], gsb[:, 0:OUT], aexp[:, 0:1]
                        )
                    else:
                        nc.vector.scalar_tensor_tensor(
                            nacc[:],
                            gsb[:, OUT * f : OUT * (f + 1)],
                            aexp[:, f : f + 1],
                            acc[:],
                            mybir.AluOpType.mult,
                            mybir.AluOpType.add,
                        )
                    acc = nacc
                if NCH < K:
                    NJ = K - NCH
                    ybuf = ybufp.tile([128, NJ * OUT], mybir.dt.float32, tag="ybuf")
                    yb_jo = ybuf[:].rearrange("p (o j) -> p j o", j=NJ)
                    for j, f in enumerate(range(NCH, K)):
                        nc.scalar.activation(
                            yb_jo[:, j : j + 1, :],
                            gsb[:, OUT * f : OUT * (f + 1)],
                            mybir.ActivationFunctionType.Copy,
                            scale=aexp[:, f : f + 1],
                        )
                    red = accp.tile([128, OUT], mybir.dt.float32, tag="red")
                    nc.vector.tensor_reduce(
                        red[:],
                        ybuf[:].rearrange("p (o j) -> p o j", j=NJ),
                        mybir.AxisListType.X,
                        mybir.AluOpType.add,
                    )
                    acc2 = accp.tile([128, OUT], mybir.dt.float32, tag="red")
                    nc.vector.tensor_add(acc2[:], acc[:], red[:])
                    acc = acc2
                # softplus(x) = relu(x) + ln(1 + exp(-min(|x|, 87))); no
                # Softplus LUT in this toolchain's act tables, but abs/exp/
                # ln/relu/copy all live in one table (natural_log_exp).
                t_abs = outp.tile([128, OUT], mybir.dt.float32, tag="outp")
                nc.scalar.activation(
                    t_abs[:], acc[:], mybir.ActivationFunctionType.Abs
                )
                t_cl = outp.tile([128, OUT], mybir.dt.float32, tag="outp")
                nc.vector.tensor_scalar_min(t_cl[:], t_abs[:], 87.0)
                t_exp = outp.tile([128, OUT], mybir.dt.float32, tag="outp")
                nc.scalar.activation(
                    t_exp[:], t_cl[:], mybir.ActivationFunctionType.Exp, scale=-1.0
                )
                t_ln = outp.tile([128, OUT], mybir.dt.float32, tag="outp")
                nc.scalar.activation(
                    t_ln[:], t_exp[:], mybir.ActivationFunctionType.Ln, bias=1.0
                )
                t_relu = outp.tile([128, OUT], mybir.dt.float32, tag="outp")
                nc.scalar.activation(
                    t_relu[:], acc[:], mybir.ActivationFunctionType.Relu
                )
                t_s = outp.tile([128, OUT], mybir.dt.float32, tag="outp")
                nc.vector.tensor_add(t_s[:], t_ln[:], t_relu[:])
                ot = outp.tile([128, OUT], mybir.dt.float32, tag="outp")
                nc.vector.tensor_add(ot[:], t_s[:], biassb[:])
                nc.gpsimd.dma_start(out_d[128 * blk : 128 * (blk + 1), :], ot[:])

    nc.compile()
    return nc


def _prep_inputs(dist_adj, dist_exp, atom_emb, bilinear_w, bilinear_b):
    dist_adj = np.asarray(dist_adj, dtype=np.float32)
    dist_exp = np.asarray(dist_exp, dtype=np.float32)
    atom_emb = np.asarray(atom_emb, dtype=np.float32)
    bilinear_w = np.asarray(bilinear_w, dtype=np.float32)
    bilinear_b = np.asarray(bilinear_b, dtype=np.float32)

    # [core, t, p, mq]: groups of 4 atoms; partition p's 4 KiB (4 atoms x
    # 1 KiB... bf16: 4x2KiB=8KiB) contiguous per DMA.  q = 64c + k, n = 16p+c.
    exp_b = (
        dist_exp.astype(_BF)
        .reshape(N_CORES, M // 4, 4, 128, 1024)
        .transpose(0, 1, 3, 2, 4)
        .reshape(N_CORES, M // 4, 128, 4096)
    )
    # adjA[core, j, 16m + c] = dist_adj[core*M + m, 16j + c]
    adjA = (
        dist_adj.reshape(N_CORES, M, 128, 16)
        .transpose(0, 2, 1, 3)
        .reshape(N_CORES, 128, 16 * M)
        .astype(_BF, order="C")
    )
    embT = atom_emb.reshape(N_CORES, M, H).transpose(0, 2, 1).astype(_BF, order="C")
    w2 = bilinear_w.transpose(1, 0, 2).reshape(H, K * OUT).astype(_BF, order="C")
    biasb = np.ascontiguousarray(
        np.broadcast_to(bilinear_b.astype(np.float32), (128, OUT))
    )

    in_maps = []
    for i in range(N_CORES):
        in_maps.append(
            {
                "exp": np.ascontiguousarray(exp_b[i]),
                "adjA": np.ascontiguousarray(adjA[i]),
                "embT": np.ascontiguousarray(embT[i]),
                "w": w2,
                "bias": biasb,
            }
        )
    return in_maps


def _run(in_maps, **kwargs):
    _ensure_path()
    from concourse.bass_utils import run_bass_kernel_spmd

    if "nc" not in _CACHE:
        _CACHE["nc"] = _build()
    nc = _CACHE["nc"]
    res = run_bass_kernel_spmd(nc, in_maps, core_ids=list(range(N_CORES)), **kwargs)
    return res


def kernel(dist_adj, dist_exp, atom_emb, bilinear_w, bilinear_b):
    in_maps = _prep_inputs(dist_adj, dist_exp, atom_emb, bilinear_w, bilinear_b)
    res = _run(in_maps)
    out = np.concatenate(
        [np.asarray(res.results[i]["out"]) for i in range(N_CORES)], axis=0
    )
    return out.astype(np.float32)

